# revision 19
# baseline (speedup 1.0000x reference)
"""CogVideoX spatial+temporal attention block on 8 Trainium2 NeuronCores.

Strategy:
  Pass 1 (spatial attention): data-parallel over the 32 frames (B*T), 4 frames
  per core, full attention over the 576 tokens of each frame.
  Pass 2 (causal temporal attention): data-parallel over the 1152 pixels
  (B*HW), 144 pixels per core; per pixel, causal attention over T=16 frames.
  Pixels are batched 8-per-128-token-group with a block-diagonal causal mask.
  The host reshards (transposes) between the passes.

  Fully SBUF-resident block pipeline: for each attention block (one frame /
  one 16-pixel group), the QKV projections are computed straight into SBUF
  block buffers (q/k in a [128, 16, TN] head-major tile, v in the AV-ready
  [128, g, 16, 65] tile with a ones-column for the softmax denominator);
  attention and the output projection then run without any DRAM round-trip.
  Block buffers are triple-buffered so block b's (scalar-engine-bound)
  softmax overlaps with block b+1's QKV matmul stream, keeping the PE dense
  and the HAM clock gate open.  The temporal causal mask is a 0/1 multiply
  on the exp'd scores (vector engine) rather than an additive mask matmul.
  The residual is taken from the resident bf16 input tile.
"""

import contextlib
import ctypes
import sys
import types

sys.path.insert(0, "/opt/trn_rl_repo")

import ml_dtypes  # noqa: E402
import numpy as np  # noqa: E402

import concourse.bass as bass  # noqa: E402
import concourse.mybir as mybir  # noqa: E402
import concourse.tile as tile  # noqa: E402

F32 = mybir.dt.float32
BF16 = mybir.dt.bfloat16
AF = mybir.ActivationFunctionType
ALU = mybir.AluOpType
NPBF16 = ml_dtypes.bfloat16

B, T, H, W, D = 2, 16, 24, 24, 1024
HW = H * W            # 576
NH, HD = 16, 64
NCORES = 8
TOK = 2304            # tokens per core, both passes

# ---------------------------------------------------------------------------
# Environment shims for this container
# ---------------------------------------------------------------------------

def _install_env_shims():
    # 1) NTFF profile hook: trn_boot only registers it when the image's
    #    antenv package has an axon_hooks module; fabricate one.
    import antenv
    if "antenv.axon_hooks" not in sys.modules:
        mod = types.ModuleType("antenv.axon_hooks")
        hook_box = [None]
        mod.set_axon_ntff_profile_hook = lambda h: hook_box.__setitem__(0, h)
        mod.get_axon_ntff_profile_hook = lambda: hook_box[0]
        sys.modules["antenv.axon_hooks"] = mod
        antenv.axon_hooks = mod
        try:
            lib = ctypes.CDLL("/opt/axon/libaxon_pjrt.so")
            if hasattr(lib, "axon_start_nrt_profile"):
                lib.axon_start_nrt_profile.argtypes = [
                    ctypes.POINTER(ctypes.c_int64), ctypes.c_size_t]
                lib.axon_start_nrt_profile.restype = ctypes.c_int64
                lib.axon_stop_nrt_profile.argtypes = [ctypes.c_char_p]
                lib.axon_stop_nrt_profile.restype = ctypes.c_int64

                @contextlib.contextmanager
                def _hook(output_dir, device_ids):
                    import jax
                    jax.devices()
                    if device_ids:
                        ids = (ctypes.c_int64 * len(device_ids))(*device_ids)
                        rc = lib.axon_start_nrt_profile(ids, len(device_ids))
                    else:
                        rc = lib.axon_start_nrt_profile(None, 0)
                    if rc != 0:
                        raise RuntimeError(f"axon_start_nrt_profile rc={rc}")
                    try:
                        yield
                    finally:
                        lib.axon_stop_nrt_profile(str(output_dir).encode())

                mod.set_axon_ntff_profile_hook(_hook)
        except OSError:
            pass
    # 2) No bucket access in this container; keep profile artifacts local.
    from concourse import bass_utils
    bass_utils.upload_artifacts = lambda tmpdir: f"local:{tmpdir}"


_install_env_shims()


def _split_sync_waits(nc):
    """This container's walrus build rejects instructions carrying more than
    one sync-wait command; hoist excess waits onto NoOps inserted immediately
    before the instruction on the same engine."""
    n = 0
    for bb in nc.main_func.blocks:
        new_insts = []
        for inst in bb.instructions:
            si = inst.sync_info
            waits = list(si.on_wait) if (si and si.on_wait) else []
            if len(waits) > 1:
                si.on_wait.clear()
                for w in waits[:-1]:
                    nop = mybir.InstNoOp(
                        name=f"{inst.name}-ws{n}",
                        engine=inst.engine,
                        sync_info=mybir.SyncInfo(on_wait=[w], on_update=[]),
                        bass_nofuse=True,
                    )
                    n += 1
                    nc.register_instruction(nop)
                    new_insts.append(nop)
                si.on_wait.append(waits[-1])
            new_insts.append(inst)
        bb.instructions[:] = new_insts
    return n


# ---------------------------------------------------------------------------
# Kernel builder (shared by the spatial and temporal passes)
# ---------------------------------------------------------------------------

def _build_pass(mode):
    assert mode in ("spatial", "temporal")
    temporal = mode == "temporal"
    nc = bass.Bass()

    F8 = mybir.dt.float8e4
    xtb = nc.declare_dram_parameter("xtb", [D, TOK], BF16, isOutput=False)
    xtb8 = nc.declare_dram_parameter("xtb8", [D, TOK], F8, isOutput=False)
    wqk = nc.declare_dram_parameter(
        "wqk", [D, 2 * D], F8 if mode == "spatial" else BF16, isOutput=False)
    wv = nc.declare_dram_parameter(
        "wv", [D, D], F8 if mode == "spatial" else BF16, isOutput=False)
    wp = nc.declare_dram_parameter("wp", [128, 8, D], BF16, isOutput=False)
    qkb = nc.declare_dram_parameter("qkb", [128, 16], F32, isOutput=False)
    vb = nc.declare_dram_parameter("vb", [128, D], BF16, isOutput=False)
    pb = nc.declare_dram_parameter("pb", [128, 8], F32, isOutput=False)
    sel = nc.declare_dram_parameter("sel", [8, 8, 64], BF16, isOutput=False)
    if temporal:
        # 0/1 causal mask, replicated across the 4 (head, group) score slots.
        mask4 = nc.declare_dram_parameter("mask4", [128, 4, 128], BF16,
                                          isOutput=False)
    out = nc.declare_dram_parameter("out", [D, TOK], F32, isOutput=True)
    # denominator-row bounce buffer (SBUF partition moves need quadrant-
    # aligned start partitions, so collect rows via DRAM instead)
    dn_d = nc.dram_tensor("dn_d", [2, 8, 1024], BF16)

    if temporal:
        TN, NB = 256, 9
        KC = [(0, 128), (128, 128)]
        SI = [(0, 256)]
    else:
        TN, NB = 576, 4
        KC = [(0, 128), (128, 128), (256, 128), (384, 128), (512, 64)]
        SI = [(0, 288), (288, 288)]
    NKC, NSI = len(KC), len(SI)

    with tile.TileContext(nc) as tc:
        with (
            tc.tile_pool(name="sb", bufs=1) as pool,
            tc.tile_pool(name="ps", bufs=1, space="PSUM") as psum,
        ):
            # ---------------- resident tiles ----------------
            wqk_r = wqk.rearrange("(c p) m -> p c m", p=128)
            xtb_r = xtb.rearrange("(c p) t -> p c t", p=128)
            xtb8_r = xtb8.rearrange("(c p) t -> p c t", p=128)
            out_r = out.rearrange("(c p) t -> p c t", p=128)

            if temporal:
                wqk_t = pool.tile([128, 8, 2 * D], BF16, tag="wqkt",
                                  name="wqkt", bufs=1)
                nc.sync.dma_start(wqk_t[:], wqk_r[:])
            wv_t = pool.tile([128, 8, D], F8 if not temporal else BF16,
                             tag="wv", name="wv", bufs=1)
            nc.sync.dma_start(wv_t[:], wv.rearrange("(c p) m -> p c m", p=128))
            vb_t = pool.tile([128, D], BF16, tag="vb", name="vb", bufs=1)
            nc.sync.dma_start(vb_t[:], vb[:])
            qkb_t = pool.tile([128, 16], F32, tag="qkb", name="qkb", bufs=1)
            nc.sync.dma_start(qkb_t[:], qkb[:])
            wp_t = pool.tile([128, 8, D], BF16, tag="wp", name="wp", bufs=1)
            nc.sync.dma_start(wp_t[:], wp[:])
            pb_t = pool.tile([128, 8], F32, tag="pb", name="pb", bufs=1)
            nc.sync.dma_start(pb_t[:], pb[:])
            ones_t = pool.tile([128, 64], BF16, tag="ones", name="ones",
                               bufs=1)
            nc.any.memset(ones_t[:], 1.0)
            # sel[p, j, :] = (p == j): stationary selector that broadcasts
            # row j of a [8, q] tile to 64 output partitions.
            sel_t = pool.tile([8, 8, 64], BF16, tag="sel", name="sel", bufs=1)
            nc.sync.dma_start(sel_t[:], sel[:])
            if temporal:
                mask4_t = pool.tile([128, 4, 128], BF16, tag="mask4",
                                    name="mask4", bufs=1)
                nc.sync.dma_start(mask4_t[:], mask4[:])

            # Per-block SBUF buffer rings (block b's softmax overlaps
            # block b+1's QKV stream and block b-1's projection).
            blk = {}
            havs = {}

            # ---------------- per-block QKV pieces ----------------
            def init_piece(b):
                t0 = b * TN
                xtb_t = pool.tile([128, 8, TN], BF16, tag="xtb",
                                  name="xtb", bufs=3)
                nc.sync.dma_start(xtb_t[:], xtb_r[:, :, t0:t0 + TN])
                if temporal:
                    xtb8_t = None
                else:
                    xtb8_t = pool.tile([128, 8, TN], F8, tag="xtb8",
                                       name="xtb8", bufs=3)
                    nc.sync.dma_start(xtb8_t[:], xtb8_r[:, :, t0:t0 + TN])
                qkt_f = pool.tile([128, 16, TN], BF16, tag="qktf",
                                  name="qktf", bufs=3)
                vx = pool.tile([128, NKC, 16, 65], BF16, tag="vx",
                               name="vx", bufs=3)
                blk[b] = [qkt_f, vx, xtb_t, None, None, xtb8_t]
                nc.gpsimd.tensor_copy(
                    vx[:, :, :, 64:65],
                    ones_t[:, 0:1][:, None, None, :].to_broadcast(
                        (128, NKC, 16, 1)))

            def co_piece(b, co):
                qkt_f, vx, xtb_t = blk[b][:3]
                xtb8_t = blk[b][5]
                if temporal:
                    wblk = wqk_t[:, :, co * 128:(co + 1) * 128]
                    for q0, qn in SI:
                        ps = psum.tile([128, 512], F32, tag="qp", name="qp",
                                       bufs=2)[:, 0:qn]
                        for ci in range(8):
                            nc.tensor.matmul(
                                ps, wblk[:, ci, :],
                                xtb_t[:, ci, q0:q0 + qn],
                                start=(ci == 0), stop=(ci == 7))
                        nc.vector.tensor_scalar_add(
                            qkt_f[:, co, q0:q0 + qn], ps,
                            qkb_t[:, co:co + 1])
                else:
                    wblk = pool.tile([128, 8, 128], F8, tag="wqkblk",
                                     name="wqkblk", bufs=3)
                    nc.sync.dma_start(wblk[:],
                                      wqk_r[:, :, co * 128:(co + 1) * 128])
                    for q0, qn in ((0, 512), (512, 64)):
                        ps = psum.tile([128, 512], F32, tag="qp", name="qp",
                                       bufs=2)[:, 0:qn]
                        if qn == 512:
                            for ci in range(0, 8, 2):
                                nc.tensor.matmul(
                                    ps, wblk[:, ci:ci + 2, :],
                                    xtb8_t[:, ci:ci + 2, q0:q0 + qn],
                                    start=(ci == 0), stop=(ci == 6),
                                    perf_mode=mybir.MatmulPerfMode.DoubleRow)
                        else:
                            for ci in range(8):
                                nc.tensor.matmul(
                                    ps, wblk[:, ci, :],
                                    xtb8_t[:, ci, q0:q0 + qn],
                                    start=(ci == 0), stop=(ci == 7))
                        nc.vector.tensor_scalar_add(
                            qkt_f[:, co, q0:q0 + qn], ps,
                            qkb_t[:, co:co + 1])

            def v_piece(b, ki, vc):
                qkt_f, vx, xtb_t = blk[b][:3]
                xtb8_t = blk[b][5]
                k0, kn = KC[ki]
                ps = psum.tile([128, 512], F32, tag="qp", name="qp",
                               bufs=2)[0:kn, :]
                if temporal:
                    for ci in range(8):
                        nc.tensor.matmul(
                            ps, xtb_t[:, ci, k0:k0 + kn],
                            wv_t[:, ci, vc * 512:(vc + 1) * 512],
                            start=(ci == 0), stop=(ci == 7))
                    nc.vector.tensor_add(
                        vx[0:kn, ki, vc * 8:(vc + 1) * 8, 0:64],
                        ps.rearrange("p (h e) -> p h e", e=64),
                        vb_t[0:kn, vc * 512:(vc + 1) * 512]
                        .rearrange("p (h e) -> p h e", e=64))
                else:
                    for ci in range(0, 8, 2):
                        nc.tensor.matmul(
                            ps, xtb8_t[:, ci:ci + 2, k0:k0 + kn],
                            wv_t[:, ci:ci + 2, vc * 512:(vc + 1) * 512],
                            start=(ci == 0), stop=(ci == 6),
                            perf_mode=mybir.MatmulPerfMode.DoubleRow)
                    nc.vector.scalar_tensor_tensor(
                        vx[0:kn, ki, vc * 8:(vc + 1) * 8, 0:64],
                        ps.rearrange("p (h e) -> p h e", e=64),
                        1.0 / 256.0,
                        vb_t[0:kn, vc * 512:(vc + 1) * 512]
                        .rearrange("p (h e) -> p h e", e=64),
                        op0=ALU.mult, op1=ALU.add)

            # ---------------- attention pieces ----------------
            def attn_start(b):
                if temporal:
                    attn2 = pool.tile([128, 8, 2, 128], BF16, tag="attn2",
                                      name="attn2", bufs=2)
                    stage = pool.tile([64, 8, 2, 128], BF16, tag="stage",
                                      name="stage", bufs=1)
                else:
                    attn2 = pool.tile([128, 8, NSI, 288], BF16, tag="attn2",
                                      name="attn2", bufs=2)
                    stage = pool.tile([64, 8, NSI, 288], BF16, tag="stage",
                                      name="stage", bufs=1)
                blk[b][3] = attn2
                blk[b][4] = stage

            def head_piece_t(b, hp):
                qkt_f, vx, xtb_t, attn2, stage = blk[b][:5]
                # e=0 / e=1 score pairs go to different PSUM banks so the
                # row-group-concurrent matmuls never co-drain into one bank.
                sps = psum.tile([128, 2, 2, 256], F32, tag="sp",
                                name="sp", bufs=2)
                av = psum.tile([65, 4, 128], F32, tag="av", name="av",
                               bufs=2)
                for e in range(2):
                    lo = e * 64
                    qT = qkt_f[lo:lo + 64, hp, :]
                    kT = qkt_f[lo:lo + 64, 8 + hp, :]
                    for gi in range(2):
                        g0 = gi * 128
                        nc.tensor.matmul(sps[:, e, gi, 0:128],
                                         kT[:, g0:g0 + 128],
                                         qT[:, g0:g0 + 128],
                                         start=True, stop=True)
                ekm = pool.tile([128, 2, 2, 128], BF16, tag="ek",
                                name="ek", bufs=3)
                nc.scalar.activation(ekm[:], sps[:, :, :, 0:128], AF.Exp,
                                     scale=0.125)
                nc.gpsimd.tensor_mul(
                    ekm[:], ekm[:],
                    mask4_t.rearrange("p (e g) q -> p e g q", e=2))
                for e in range(2):
                    h = 2 * hp + e
                    for gi in range(2):
                        s = 2 * e + gi
                        nc.tensor.matmul(av[0:65, s, :], vx[:, gi, h, :],
                                         ekm[:, e, gi, :],
                                         start=True, stop=True)
                avs = pool.tile([65, 4, 128], BF16, tag="avs", name="avs",
                                bufs=4)
                nc.vector.tensor_copy(avs[:], av[0:65, :, :])
                lgt = pool.tile([65, 4, 128], F32, tag="lgt",
                                name="lgt", bufs=1)
                nc.scalar.activation(lgt[64:65, :, :], avs[64:65, :, :],
                                     AF.Ln)
                dcp = pool.tile([65, 4, 128], BF16, tag="dcp",
                                name="dcp", bufs=2)
                nc.scalar.activation(dcp[64:65, :, :], lgt[64:65, :, :],
                                     AF.Exp, scale=-1.0)
                bc = psum.tile([64, 4, 128], F32, tag="qp", name="qp",
                               bufs=2)
                nc.tensor.matmul(bc[0:64, :, :], ones_t[64:65, 0:64],
                                 dcp[64:65, :, :], start=True, stop=True)
                bcs = pool.tile([64, 4, 128], BF16, tag="bcs",
                                name="bcs", bufs=2)
                nc.vector.tensor_copy(bcs[:], bc[0:64, :, :])
                nc.vector.tensor_mul(attn2[0:64, hp],
                                     avs[0:64, 0:2, :], bcs[:, 0:2, :])
                nc.vector.tensor_mul(stage[:, hp],
                                     avs[0:64, 2:4, :], bcs[:, 2:4, :])
                if hp == 3:
                    nc.sync.dma_start(attn2[64:128, 0:4], stage[:, 0:4])
                elif hp == 7:
                    nc.sync.dma_start(attn2[64:128, 4:8], stage[:, 4:8])

            def tail_piece_t(b, half):
                pass

            def head_piece_s(b, h):
                qkt_f, vx, xtb_t, attn2, stage = blk[b][:5]
                c, lo = h // 2, (h % 2) * 64
                qT = qkt_f[lo:lo + 64, c, :]
                kT = qkt_f[lo:lo + 64, 8 + c, :]
                av = psum.tile([65, 2, 512], F32, tag="av", name="av",
                               bufs=1)
                for ki, (k0, kn) in enumerate(KC):
                    sps = psum.tile([128, 2, 512], F32, tag="sp",
                                    name="sp", bufs=2)
                    for si, (q0, qn) in enumerate(SI):
                        nc.tensor.matmul(
                            sps[0:kn, si, 0:qn], kT[:, k0:k0 + kn],
                            qT[:, q0:q0 + qn], start=True, stop=True)
                    ek = pool.tile([128, 2, 288], BF16, tag="ek",
                                   name="ek", bufs=2)
                    nc.scalar.activation(ek[0:kn], sps[0:kn, :, 0:288],
                                         AF.Exp, scale=0.125 / 65536.0)
                    for si, (q0, qn) in enumerate(SI):
                        nc.tensor.matmul(
                            av[0:65, si, 0:qn], vx[0:kn, ki, h, :],
                            ek[0:kn, si, :],
                            start=(ki == 0), stop=(ki == NKC - 1))
                avs = pool.tile([65, 2, 288], BF16, tag="avs", name="avs",
                                bufs=8)
                nc.vector.tensor_copy(avs[:], av[0:65, :, 0:288])
                nc.sync.dma_start(
                    dn_d[h // 8, h % 8, 0:576]
                    .rearrange("(s q) -> s q", s=2)[None, :, :],
                    avs[64:65, :, :])
                havs[(b, h)] = avs

            def tail_piece_s(b, half):
                qkt_f, vx, xtb_t, attn2, stage = blk[b][:5]
                coll_t = pool.tile([8, 2, 288], BF16, tag="coll",
                                   name="coll", bufs=2)
                nc.sync.dma_start(
                    coll_t[:],
                    dn_d[half, 0:8, 0:576]
                    .rearrange("h (s q) -> h s q", s=2))
                lgt = pool.tile([8, 2, 288], F32, tag="lgt",
                                name="lgt", bufs=1)
                nc.scalar.activation(lgt[:], coll_t[:], AF.Ln)
                dcpb = pool.tile([8, 2, 288], BF16, tag="dcp",
                                 name="dcp", bufs=2)
                nc.scalar.activation(dcpb[:], lgt[:], AF.Exp, scale=-1.0)
                for h in range(half * 8, half * 8 + 8):
                    avs = havs.pop((b, h))
                    c = h // 2
                    bc = psum.tile([128, 2, 512], F32, tag="sp",
                                   name="sp", bufs=2)
                    for si in range(NSI):
                        nc.tensor.matmul(bc[0:64, si, 0:288],
                                         sel_t[:, h % 8, :],
                                         dcpb[:, si, :],
                                         start=True, stop=True)
                    bcs = pool.tile([64, NSI, 288], BF16, tag="bcs",
                                    name="bcs", bufs=2)
                    nc.vector.tensor_copy(bcs[:], bc[0:64, :, 0:288])
                    dst = stage[:, c] if (h % 2) else attn2[0:64, c]
                    nc.vector.tensor_mul(dst, avs[0:64, :, :], bcs[:])
                if half == 0:
                    nc.sync.dma_start(attn2[64:128, 0:4], stage[:, 0:4])
                else:
                    nc.sync.dma_start(attn2[64:128, 4:8], stage[:, 4:8])

            # ---------------- projection pieces ----------------
            def proj_piece(b, dc):
                t0 = b * TN
                qkt_f, vx, xtb_t, attn2, stage = blk[b][:5]
                if temporal:
                    pps = psum.tile([128, 512], F32, tag="qp",
                                    name="qp", bufs=2)
                    for p in range(8):
                        nc.tensor.matmul(
                            pps[:, 0:256],
                            wp_t[:, p, dc * 128:(dc + 1) * 128],
                            attn2[:, p].rearrange("p s q -> p (s q)"),
                            start=(p == 0), stop=(p == 7))
                    ppv = pps[:, 0:256].rearrange("p (s q) -> p s q", s=2)
                else:
                    pps = psum.tile([128, 2, 512], F32, tag="sp",
                                    name="sp", bufs=2)
                    for p in range(8):
                        for si, (q0, qn) in enumerate(SI):
                            nc.tensor.matmul(
                                pps[:, si, 0:qn],
                                wp_t[:, p, dc * 128:(dc + 1) * 128],
                                attn2[:, p, si, :],
                                start=(p == 0), stop=(p == 7))
                    ppv = pps[:, :, 0:288]
                osb = pool.tile([128, NSI, TN // NSI], F32, tag="osb",
                                name="osb", bufs=2)
                nc.vector.scalar_tensor_tensor(
                    osb[:], ppv, pb_t[:, dc:dc + 1],
                    xtb_t[:, dc, :].rearrange("p (s q) -> p s q", s=NSI),
                    op0=ALU.add, op1=ALU.add)
                nc.sync.dma_start(
                    out_r[:, dc, t0:t0 + TN],
                    osb[:].rearrange("p s q -> p (s q)"))
                if dc == 7:
                    del blk[b]

            # ---------------- PE warm-keeper ----------------
            # Always-ready junk matmuls appended at each phase's end (lowest
            # scheduler priority): the list scheduler only slots them into PE
            # idle windows, keeping the HAM clock gate at K=8/8 through
            # scalar-engine-bound softmax stretches.
            def dummy_piece():
                ps = psum.tile([128, 512], F32, tag="qp", name="dmy",
                               bufs=2)
                for i in range(8):
                    nc.tensor.matmul(ps, wp_t[:, i, 0:128],
                                     wp_t[:, i, 0:512],
                                     start=True, stop=True)

            # ---------------- zippered emission ----------------
            def merge(lists):
                # Proportional round-robin merge of thunk lists.
                idx = [0] * len(lists)
                out = []
                total = sum(len(ls) for ls in lists)
                for _ in range(total):
                    j = min((i for i in range(len(lists))
                             if idx[i] < len(lists[i])),
                            key=lambda i: (idx[i] + 0.5) / len(lists[i]))
                    out.append(lists[j][idx[j]])
                    idx[j] += 1
                return out

            NHP = 8 if temporal else 16
            for p in range(NB + 2):
                lists = []
                if p < NB:
                    init_piece(p)
                    qkv = [(lambda b=p, co=co: co_piece(b, co))
                           for co in range(16)]
                    qkv += [(lambda b=p, ki=ki, vc=vc: v_piece(b, ki, vc))
                            for ki in range(NKC) for vc in range(2)]
                    lists.append(qkv)
                if 1 <= p <= NB:
                    attn_start(p - 1)
                    hp_f = head_piece_t if temporal else head_piece_s
                    tl_f = tail_piece_t if temporal else tail_piece_s
                    al = []
                    for h in range(NHP):
                        al.append(lambda b=p - 1, h=h: hp_f(b, h))
                        if h == NHP // 2 - 1:
                            al.append(lambda b=p - 1: tl_f(b, 0))
                        elif h == NHP - 1:
                            al.append(lambda b=p - 1: tl_f(b, 1))
                    lists.append(al)
                if 2 <= p <= NB + 1:
                    lists.append([(lambda b=p - 2, dc=dc: proj_piece(b, dc))
                                  for dc in range(8)])
                for f in merge(lists):
                    f()
                if not temporal:
                    for _ in range(12 if p >= NB else (8 if p >= 2 else 2)):
                        dummy_piece()

    _split_sync_waits(nc)
    return nc


_PROGRAMS = {}


def _get_program(mode):
    if mode not in _PROGRAMS:
        _PROGRAMS[mode] = _build_pass(mode)
    return _PROGRAMS[mode]


# ---------------------------------------------------------------------------
# Host wrapper
# ---------------------------------------------------------------------------

TRACE = False
LAST_EXEC_NS = {}
LAST_PROFILE = {}


def _run_pass(mode, xt_cores, wqkv, bqkv, wproj, bproj, mask01=None):
    from concourse.bass_utils import run_bass_kernel_spmd
    nc = _get_program(mode)
    NPF8 = ml_dtypes.float8_e4m3fn
    wqkv = np.asarray(wqkv, np.float32)
    if mode == "spatial":
        wqk = np.ascontiguousarray((wqkv[:, :2 * D] * 16.0).astype(NPF8))
    else:
        wqk = np.ascontiguousarray(wqkv[:, :2 * D].astype(NPBF16))
    if mode == "spatial":
        wv = np.ascontiguousarray((wqkv[:, 2 * D:] * 16.0).astype(NPF8))
    else:
        wv = np.ascontiguousarray(wqkv[:, 2 * D:].astype(NPBF16))
    wp_r = np.ascontiguousarray(
        np.asarray(wproj, np.float32).reshape(8, 128, D)
        .transpose(1, 0, 2).astype(NPBF16))
    bqkv = np.asarray(bqkv, np.float32)
    qksc = 256.0 if mode == "spatial" else 1.0
    qkb = np.ascontiguousarray(qksc * bqkv[:2 * D].reshape(16, 128).T)
    vb_b = np.ascontiguousarray(
        np.broadcast_to(bqkv[2 * D:], (128, D)).astype(NPBF16))
    pb = np.ascontiguousarray(np.asarray(bproj, np.float32).reshape(8, 128).T)
    in_maps = []
    for c in range(NCORES):
        m = {"xtb": np.ascontiguousarray(xt_cores[c].astype(NPBF16)),
             "xtb8": np.ascontiguousarray(
                 (xt_cores[c] * 16.0).astype(NPF8)),
             "wqk": wqk, "wv": wv, "wp": wp_r,
             "qkb": qkb, "vb": vb_b, "pb": pb,
             "sel": np.ascontiguousarray(
                 np.broadcast_to(np.eye(8, dtype=NPBF16)[:, :, None],
                                 (8, 8, 64)))}
        if mask01 is not None:
            m["mask4"] = np.ascontiguousarray(
                np.broadcast_to(
                    np.asarray(mask01, np.float32)[:, None, :],
                    (128, 4, 128)).astype(NPBF16))
        in_maps.append(m)
    res = run_bass_kernel_spmd(nc, in_maps, core_ids=list(range(NCORES)),
                               trace=TRACE)
    if TRACE:
        LAST_EXEC_NS[mode] = res.exec_time_ns
        LAST_PROFILE[mode] = res.profile_json
    return [res.results[c]["out"] for c in range(NCORES)]


def kernel(x, ws_qkv, bs_qkv, ws_proj, bs_proj, wt_qkv, bt_qkv, wt_proj,
           bt_proj, T=T, H=H, W=W, **_kw):
    x = np.asarray(x, np.float32)
    # ---- pass 1: spatial, shard over frames --------------------------------
    xT = np.ascontiguousarray(x.reshape(B * T * HW, D).T)   # (D, 18432)
    xt_cores = [xT[:, c * 4 * HW:(c + 1) * 4 * HW] for c in range(NCORES)]
    outs = _run_pass("spatial", xt_cores, ws_qkv, bs_qkv, ws_proj, bs_proj)
    x1T = np.concatenate(outs, axis=1)                      # (D, 18432)
    # ---- reshard: (D, B, T, HW) -> (D, B, HW, T), shard over pixels --------
    x1p = np.ascontiguousarray(
        x1T.reshape(D, B, T, HW).transpose(0, 1, 3, 2).reshape(D, B * HW * T))
    kk, qq = np.meshgrid(np.arange(128), np.arange(128), indexing="ij")
    mask01 = (((kk // 16) == (qq // 16)) & ((kk % 16) <= (qq % 16))
              ).astype(np.float32)
    xt_cores2 = [x1p[:, c * TOK:(c + 1) * TOK] for c in range(NCORES)]
    outs2 = _run_pass("temporal", xt_cores2, wt_qkv, bt_qkv, wt_proj, bt_proj,
                      mask01)
    x2p = np.concatenate(outs2, axis=1)
    out = x2p.reshape(D, B, HW, T).transpose(0, 1, 3, 2).reshape(D, B * T * HW)
    return np.ascontiguousarray(out.T).reshape(B, T * HW, D)
